# revision 13
# baseline (speedup 1.0000x reference)
"""Trainium2 Bass kernel for DecoderAttention (Luong attention).

reference:
    query   = dec_out @ W.T                    # (B, P, D)
    scores  = query @ enc_out.T (per batch)    # (B, P, S)
    scores  = where(mask, -inf, scores)
    weight  = softmax(scores, -1)
    context = weight @ enc_out                 # (B, P, D)

B=256, S=512, P=128, D=512 (fp32 I/O). Data-parallel over 8 NeuronCores
(32 batches per core). All matmuls fp16 on the PE (1 cycle/row vs 4 for
fp32; PSUM accumulates fp32); inputs cast to fp16 on host, output stored
fp16 and upcast on host.

Mask sparsity: masked positions get softmax weight exactly 0, so the
host gathers only the unmasked enc rows per batch (zero-padding to the
slot width w, a multiple of 8). Zero rows contribute exp(0-max) ~ e^-60
to the softmax denominator (invisible in fp32) and exactly 0 to the
context. Batches are sorted by unmasked count and dealt round-robin
across the 8 cores so one SPMD program serves all cores; output is
scattered back on host.

v3 structure:
  - Per-slot enc data packed into ONE dram tensor, per-partition layout
    [4*w enc^T (d-major)][(kt-1)*512 enc s-major full chunks][512
    remainder chunk], loaded as two contiguous DMAs (full-partition main
    rect on the sync queue, remainder rows on the scalar queue). No
    zero-padding traffic beyond pad-to-8.
  - Tails (weight transpose + context matmul + store) lag TWO slots
    behind the scores matmul, so the PE never waits on the
    mm2 -> reduce_max -> exp cross-engine chain. PE order per iter is
    transpose(b-2), mm2(b), mm3(b-2): the weight^T PSUM->SBUF copy
    (DVE) hides under mm2.
  - mm1 is spread one em-chain per iteration: group g+1's query^T is
    computed during group g's four iterations, one 4-matmul chain +
    one PSUM->SBUF cast each (casts alternate DVE/ACT), so no
    iteration carries a 2.7us cast burst.
  - PE warm-up: ~18 dummy matmuls on a memset tile raise the PE from
    its cold p-state (0.65/1.2 GHz) to 2.4 GHz while the first input
    DMAs are still in flight.
  - dect packed per-group contiguous ([128, 4*512] lines of 4KB),
    prefetched 6 iterations ahead; enc prefetched 3 slots ahead.

Per-core layout (K = PE contraction dim = partition dim):
  mm1  query^T (e,p): lhsT = W^T tiles (d,e) [stationary, shared],
       rhs = dec^T packed 4 slots (d, 4*128) -> N=512 moving.
  mm2  scores (p,s'): lhsT = query^T tiles, rhs = gathered enc^T tiles.
  softmax: DVE reduce_max (negate) -> ACT exp(bias=-max, accum_out=sum)
       -> DVE reciprocal; 1/sum applied by ACT during the context
       PSUM->SBUF copy (activation Copy, scale per partition).
  mm3  context (p,d): lhsT = weight^T (PE transposes), rhs = enc rows;
       last k-tile runs with K = remainder rows (never reads the
       unwritten pad partitions).
"""

import sys
import types

import numpy as np

B, SRC, PRED, D = 256, 512, 128, 512
N_CORES = 8
NB = B // N_CORES  # batches per core
TRIM_TAIL = True

MIN_W = 32
PREFETCH = 3  # enc slots in flight ahead of use
TAIL_LAG = 2
N_WARMUP = 18  # dummy matmuls to raise the PE p-state before real work


# ---------------------------------------------------------------------------
# environment shims (walrus 1-wait/instruction limit; missing axon hooks)
# ---------------------------------------------------------------------------
def _install_fixes():
    import concourse.tile as tile
    from concourse.tile import ScopedClock
    from concourse import mybir, bass_utils

    if not getattr(tile.TileContext, "_drain_split_installed", False):

        def _drain_and_barrier(self, tick_clock, wait_clock):
            nc = self.nc
            drain_inst = nc.sync.drain()
            wait_clock.add_sem_waits(
                drain_inst.ins, ScopedClock({None: tick_clock.global_clock})
            )
            waits = list(drain_inst.ins.sync_info.on_wait)
            if len(waits) > 1:
                drain_inst.ins.sync_info.on_wait = waits[:1]
                for w in waits[1:]:
                    extra = nc.sync.drain()
                    extra.ins.sync_info = mybir.SyncInfo(on_wait=[w], on_update=[])
            assert self.sems is not None
            popped = nc._tile_sem_poison_stack.pop()
            assert popped is self._sem_poison
            if not TRIM_TAIL:
                nc.all_engine_barrier()
                nc.clear_and_free_semaphores(list(self.sems.allocated().values()))
                nc.all_engine_barrier()
            # TRIM_TAIL: single execution per NEFF — skip the sem-clear
            # butterfly and barriers entirely (handles leak, harmless).

        tile.TileContext._drain_and_barrier = _drain_and_barrier
        tile.TileContext._drain_split_installed = True

    try:
        import antenv.axon_hooks  # noqa: F401
    except ImportError:
        try:
            if "/root/.axon_site" not in sys.path:
                sys.path.insert(0, "/root/.axon_site")
            from trn_agent_boot.trn_boot import _ntff_profile_via_ctypes

            hook = _ntff_profile_via_ctypes("/opt/axon/libaxon_pjrt.so")
            mod = types.ModuleType("antenv.axon_hooks")
            mod._hook = hook
            mod.get_axon_ntff_profile_hook = lambda: mod._hook
            mod.set_axon_ntff_profile_hook = lambda h: setattr(mod, "_hook", h)
            sys.modules["antenv.axon_hooks"] = mod
            import antenv

            antenv.axon_hooks = mod
        except Exception:
            pass

    bass_utils.upload_artifacts = lambda tmpdir: tmpdir

    # walrus in this image accepts only ONE sync-wait per instruction; Tile
    # emits several. Split extras onto EventSemaphore wait-carriers placed
    # just before the instruction in the same engine stream (JSON-level
    # post-pass on the serialized BIR).
    import json as _json
    import concourse.bass as _bass

    if not getattr(_bass.Bass, "_waitsplit_installed", False):
        _orig_to_json = _bass.Bass.to_json_bytes

        def _split_waits(bir: bytes) -> bytes:
            m = _json.loads(bir)
            ctr = 0
            changed = False
            for f in m["functions"]:
                for bb in f["blocks"]:
                    out = []
                    for inst in bb["instructions"]:
                        si = inst.get("sync_info")
                        waits = si.get("on_wait", []) if si else []
                        if len(waits) > 1:
                            changed = True
                            for w in waits[:-1]:
                                ctr += 1
                                out.append(
                                    {
                                        "debug": inst.get("debug", 0),
                                        "engine": inst["engine"],
                                        "ins": [],
                                        "outs": [],
                                        "name": f"waitsplit_{ctr}",
                                        "opcode": "EventSemaphore",
                                        "sync_info": {
                                            "on_update": [],
                                            "on_wait": [w],
                                        },
                                    }
                                )
                            si["on_wait"] = [waits[-1]]
                        out.append(inst)
                    bb["instructions"] = out
            if not changed:
                return bir
            return _json.dumps(m).encode()

        def to_json_bytes(self, *a, **k):
            return _split_waits(_orig_to_json(self, *a, **k))

        _bass.Bass.to_json_bytes = to_json_bytes
        _bass.Bass._waitsplit_installed = True


# ---------------------------------------------------------------------------
# slot planning: sort batches by unmasked count, deal across cores
# ---------------------------------------------------------------------------
def plan_slots(attn_mask, n_cores=N_CORES):
    """Returns (assigned, widths): assigned[i, c] = source batch index for
    core c slot i; widths[i] = padded-to-8 max unmasked count in slot i."""
    attn_mask = np.asarray(attn_mask)
    n = (~attn_mask).sum(axis=1)
    order = np.argsort(-n, kind="stable")
    nb = order.size // n_cores
    assigned = order.reshape(nb, n_cores)
    widths = []
    for i in range(nb):
        w = int(n[assigned[i]].max())
        w = min(SRC, max(MIN_W, ((w + 7) // 8) * 8))
        widths.append(w)
    return assigned, widths


def slot_geom(widths):
    """Per-slot (kt, rem, L): k-tile count, rows in the last k-tile, and
    packed per-partition length. Layout: [4*w enc^T][(kt-1)*512 enc
    full chunks][512 remainder chunk (rem rows used)]."""
    kts = [(w + 127) // 128 for w in widths]
    rems = [w - 128 * (kt - 1) for w, kt in zip(widths, kts)]
    Ls = [4 * w + kt * 512 for w, kt in zip(widths, kts)]
    return kts, rems, Ls


# ---------------------------------------------------------------------------
# bass program (one NeuronCore, NB slots with per-slot widths)
# ---------------------------------------------------------------------------
def build_bass(widths, nb=NB):
    import concourse.bass as bass
    import concourse.tile as tile
    from concourse import mybir, masks
    from contextlib import ExitStack

    assert len(widths) == nb
    kts, rems, Ls = slot_geom(widths)
    ktmax = max(kts)
    Lmax = max(Ls)

    f32 = mybir.dt.float32
    f16 = mybir.dt.float16
    nc = bass.Bass()

    comb_d = nc.dram_tensor("comb", [nb, 128, Lmax], f16, kind="ExternalInput")
    dect_d = nc.dram_tensor("dect", [nb // 4, 128, 4 * 512], f16, kind="ExternalInput")
    wts_d = nc.dram_tensor("wts", [128, 4 * D], f16, kind="ExternalInput")
    out_d = nc.dram_tensor("out", [nb, PRED, D], f16, kind="ExternalOutput")

    with tile.TileContext(nc) as tc, ExitStack() as ctx:
        const = ctx.enter_context(tc.tile_pool(name="const", bufs=1))
        enc_p = ctx.enter_context(
            tc.tile_pool(name="enc", bufs=PREFETCH + TAIL_LAG + 3)
        )
        dect_p = ctx.enter_context(tc.tile_pool(name="dect", bufs=3))
        qt_p = ctx.enter_context(tc.tile_pool(name="qt", bufs=2))
        w_p = ctx.enter_context(tc.tile_pool(name="w", bufs=TAIL_LAG + 2))
        wt_p = ctx.enter_context(tc.tile_pool(name="wt", bufs=2))
        o_p = ctx.enter_context(tc.tile_pool(name="o", bufs=4))
        st_p = ctx.enter_context(tc.tile_pool(name="st", bufs=2 * (TAIL_LAG + 1)))
        ps_qt = ctx.enter_context(
            tc.tile_pool(name="ps_qt", bufs=2, space=bass.MemorySpace.PSUM)
        )
        ps_tr = ctx.enter_context(
            tc.tile_pool(name="ps_tr", bufs=2, space=bass.MemorySpace.PSUM)
        )
        ps_sc = ctx.enter_context(
            tc.tile_pool(name="ps_sc", bufs=2, space=bass.MemorySpace.PSUM)
        )
        ps_cx = ctx.enter_context(
            tc.tile_pool(name="ps_cx", bufs=2, space=bass.MemorySpace.PSUM)
        )

        ident = const.tile([128, 128], f16)
        wts_sb = const.tile([128, 4 * D], f16)

        def load_enc_main(b):
            """Main rect (enc^T + full s-chunks) on the sync queue, plus
            the pad-zeroing memset for the remainder chunk. The remainder
            DMA is issued separately (load_enc_rem) on the scalar queue at
            the END of the iteration: its memset->DMA dependency chain
            (+900ns sem prop) must never sit at a queue head in front of
            issues the pipeline needs now."""
            kt, rem, L = kts[b], rems[b], Ls[b]
            main = L - 512  # 4*w + (kt-1)*512
            enc_sb = enc_p.tile([128, Lmax], f16, tag="enc")
            nc.sync.dma_start(enc_sb[:, 0:main], comb_d[b, :, 0:main])
            if rem < 128:
                # zero the last chunk (partition-0-start access; the BIR
                # verifier rejects partition offsets); the remainder DMA
                # overwrites rows 0:rem -> pad rows stay zero and mm3 can
                # run full-K tiles
                nc.vector.memset(enc_sb[:, main:L], 0.0)
            return enc_sb

        def load_enc_rem(b, enc_sb):
            rem, L = rems[b], Ls[b]
            main = L - 512
            nc.scalar.dma_start(enc_sb[0:rem, main:L], comb_d[b, 0:rem, main:L])

        def load_dect(g):
            dect_sb = dect_p.tile([128, 4 * 512], f16, tag="dect", name="dect")
            nc.scalar.dma_start(dect_sb[:], dect_d[g])
            return dect_sb

        qt_tiles = {}
        dect_tiles = {}

        def emit_em(g, em):
            """One 4-matmul chain of group g's query^T + one PSUM cast."""
            q_ps = ps_qt.tile([128, 512], f32)
            for dk in range(4):
                nc.tensor.matmul(
                    q_ps[:],
                    wts_sb[:, dk * 512 + em * 128 : dk * 512 + (em + 1) * 128],
                    dect_tiles[g][:, dk * 512 : (dk + 1) * 512],
                    start=(dk == 0),
                    stop=(dk == 3),
                )
            dst = qt_tiles[g][:, em * 512 : (em + 1) * 512]
            if em % 2 == 0:
                nc.vector.tensor_copy(dst, q_ps[:])
            else:
                nc.scalar.activation(
                    dst, q_ps[:], mybir.ActivationFunctionType.Copy
                )

        def emit_tr(b, enc_sb, w_sb, recip):
            """weight^T via PE transposes + DVE copy back to SBUF."""
            kt, rem = kts[b], rems[b]
            wt_ps = ps_tr.tile([128, ktmax * 128], f16, tag="tr")
            for sk in range(kt):
                nc.tensor.transpose(
                    wt_ps[:, sk * 128 : (sk + 1) * 128],
                    w_sb[:, sk * 128 : (sk + 1) * 128],
                    ident[:],
                )
            wt_sb = wt_p.tile([128, ktmax * 128], f16, tag="wt")
            nc.vector.tensor_copy(wt_sb[:, 0 : kt * 128], wt_ps[:, 0 : kt * 128])
            return b, enc_sb, wt_sb, recip

        def emit_mm3(b, enc_sb, wt_sb, recip, last):
            kt, rem, w = kts[b], rems[b], widths[b]
            enc_off = 4 * w  # s-major chunks start here
            # last slot: split into halves so the first half's scale+store
            # overlaps the second half's matmuls (tail chain)
            halves = ((0, 256), (256, 512)) if last else ((0, 512),)
            o_sb = o_p.tile([128, D], f16, tag="o")
            for lo, hi in halves:
                cx_ps = ps_cx.tile([128, hi - lo], f32, tag="cx")
                for sk in range(kt):
                    nc.tensor.matmul(
                        cx_ps[:],
                        wt_sb[:, sk * 128 : (sk + 1) * 128],
                        enc_sb[:, enc_off + sk * 512 + lo : enc_off + sk * 512 + hi],
                        start=(sk == 0),
                        stop=(sk == kt - 1),
                    )
                nc.scalar.activation(
                    o_sb[:, lo:hi],
                    cx_ps[:],
                    mybir.ActivationFunctionType.Copy,
                    scale=recip[:],
                )
                nc.scalar.dma_start(out_d[b, :, lo:hi], o_sb[:, lo:hi])

        # startup: warm the PE p-state on dummy matmuls (on the weight
        # tile, first DMA to land) while the other operands stream in
        nc.sync.dma_start(wts_sb[:], wts_d[:])
        dect_tiles[0] = load_dect(0)  # scalar queue
        dect_tiles[1] = load_dect(1)
        enc_tiles = {i: load_enc_main(i) for i in range(min(PREFETCH, nb))}
        for i in range(min(PREFETCH, nb)):
            load_enc_rem(i, enc_tiles[i])
        masks.make_identity(nc, ident[:])
        for _ in range(N_WARMUP):
            dum_ps = ps_cx.tile([128, 512], f32, tag="cx")
            nc.tensor.matmul(
                dum_ps[:], wts_sb[:, 0:128], wts_sb[:, 0:512],
                start=True, stop=True,
            )
        # group 0's query^T fully in the prologue
        qt_tiles[0] = qt_p.tile([128, 4 * 512], f16, tag="qt", name="qt")
        for em in range(4):
            emit_em(0, em)
        pend = []  # softmax-done slots awaiting their tail

        for b in range(nb):
            g, j = divmod(b, 4)
            w, kt = widths[b], kts[b]

            if b + PREFETCH < nb:
                enc_tiles[b + PREFETCH] = load_enc_main(b + PREFETCH)
            if j == 2 and g + 2 < nb // 4:
                dect_tiles[g + 2] = load_dect(g + 2)

            # ---- mm1 share: em chain j of group g+1 ----------------------
            if g + 1 < nb // 4:
                if j == 0:
                    qt_tiles[g + 1] = qt_p.tile([128, 4 * 512], f16, tag="qt", name="qt")
                emit_em(g + 1, j)

            # ---- weight^T of batch b-TAIL_LAG ----------------------------
            t_tr = None
            if len(pend) == TAIL_LAG:
                t_tr = emit_tr(*pend.pop(0))

            # ---- mm2: scores (p, s') -------------------------------------
            enc_sb = enc_tiles[b]
            sc_ps = ps_sc.tile([128, w], f32, tag="sc")
            for ek in range(4):
                nc.tensor.matmul(
                    sc_ps[:],
                    qt_tiles[g][:, ek * 512 + j * 128 : ek * 512 + (j + 1) * 128],
                    enc_sb[:, ek * w : (ek + 1) * w],
                    start=(ek == 0),
                    stop=(ek == 3),
                )

            # ---- softmax -------------------------------------------------
            negmax = st_p.tile([128, 1], f32, tag="negmax")
            nc.vector.reduce_max(
                negmax[:], sc_ps[:], axis=mybir.AxisListType.X, negate=True
            )
            w_sb = w_p.tile([128, ktmax * 128], f16, tag="w")
            if w < kt * 128:
                # zero pad cols: their transposed rows pair with zero enc rows
                nc.gpsimd.memset(w_sb[:, w : kt * 128], 0.0)
            sumexp = st_p.tile([128, 1], f32, tag="sumexp")
            nc.scalar.activation(
                w_sb[:, 0:w],
                sc_ps[:],
                mybir.ActivationFunctionType.Exp,
                bias=negmax[:],
                accum_out=sumexp[:],
            )
            recip = st_p.tile([128, 1], f32, tag="recip")
            nc.vector.reciprocal(recip[:], sumexp[:])

            # ---- context + store of batch b-TAIL_LAG ---------------------
            if t_tr is not None:
                emit_mm3(*t_tr, last=False)
                del enc_tiles[t_tr[0]]

            if b + PREFETCH < nb:
                load_enc_rem(b + PREFETCH, enc_tiles[b + PREFETCH])

            pend.append((b, enc_sb, w_sb, recip))

        while pend:
            t_tr = emit_tr(*pend.pop(0))
            emit_mm3(*t_tr, last=(not pend))
            del enc_tiles[t_tr[0]]

    return nc


# ---------------------------------------------------------------------------
# host-side sharding / gather
# ---------------------------------------------------------------------------
def prepare_in_maps(enc_out, dec_out, attn_mask, W, assigned, widths,
                    n_cores=N_CORES):
    enc_out = np.asarray(enc_out, dtype=np.float32)
    dec_out = np.asarray(dec_out, dtype=np.float32)
    attn_mask = np.asarray(attn_mask)
    W = np.asarray(W, dtype=np.float32)

    nb = assigned.shape[0]
    kts, rems, Ls = slot_geom(widths)
    Lmax = max(Ls)

    wt = W.T  # (d, e)
    wts = np.ascontiguousarray(
        wt.reshape(4, 128, D).transpose(1, 0, 2).reshape(128, 4 * D)
    ).astype(np.float16)

    enc16 = enc_out.astype(np.float16)
    in_maps = []
    for c in range(n_cores):
        idx = assigned[:, c]  # source batches in slot order
        comb = np.zeros((nb, 128, Lmax), dtype=np.float16)
        for i, src in enumerate(idx):
            rows = np.flatnonzero(~attn_mask[src])
            g = enc16[src, rows]  # (w0, 512)
            w0 = rows.size
            kt, w = kts[i], widths[i]
            # enc^T section: [p, ek*w + s] = g.T[ek*128+p, s]
            t = np.zeros((D, w), dtype=np.float16)
            t[:, :w0] = g.T
            comb[i, :, : 4 * w] = (
                t.reshape(4, 128, w).transpose(1, 0, 2).reshape(128, 4 * w)
            )
            # s-major section: [p, 4*w + sk*512 + e] = g[sk*128+p, e]
            gp = np.zeros((kt * 128, D), dtype=np.float16)
            gp[:w0] = g
            comb[i, :, 4 * w : 4 * w + kt * 512] = (
                gp.reshape(kt, 128, D).transpose(1, 0, 2).reshape(128, kt * 512)
            )
        dec_c = dec_out[idx]  # (nb, P, D)
        dect = np.ascontiguousarray(
            dec_c.reshape(nb // 4, 4, PRED, D)  # (g, jslot, p, d)
            .transpose(0, 3, 1, 2)              # (g, d, jslot, p)
            .reshape(nb // 4, 4, 128, 4 * PRED)  # (g, dk, dp, n)
            .transpose(0, 2, 1, 3)              # (g, dp, dk, n)
            .reshape(nb // 4, 128, 4 * 512)
        ).astype(np.float16)
        in_maps.append({"comb": comb, "dect": dect, "wts": wts})
    return in_maps


def run_sharded(enc_out, dec_out, attn_mask, W, trace=False, trace_kwargs=None):
    """Returns (full_output, BassKernelResults)."""
    _install_fixes()
    from concourse import bass_utils

    attn_mask = np.asarray(attn_mask)
    assigned, widths = plan_slots(attn_mask)
    nc = build_bass(widths)
    in_maps = prepare_in_maps(enc_out, dec_out, attn_mask, W, assigned, widths)
    res = bass_utils.run_bass_kernel_spmd(
        nc,
        in_maps,
        list(range(N_CORES)),
        trace=trace,
        **(trace_kwargs or {}),
    )
    out = np.empty((B, PRED, D), dtype=np.float32)
    for c in range(N_CORES):
        out[assigned[:, c]] = res.results[c]["out"].astype(np.float32)
    return out, res


def kernel(enc_out, dec_out, attn_mask, W):
    out, _ = run_sharded(enc_out, dec_out, attn_mask, W, trace=False)
    return out.astype(np.float32)


if __name__ == "__main__":
    print("building bass program...")
    _install_fixes()
    nc = build_bass([264] * NB)
    print("ok")


# revision 14
# speedup vs baseline: 1.0365x; 1.0365x over previous
"""Trainium2 Bass kernel for DecoderAttention (Luong attention).

reference:
    query   = dec_out @ W.T                    # (B, P, D)
    scores  = query @ enc_out.T (per batch)    # (B, P, S)
    scores  = where(mask, -inf, scores)
    weight  = softmax(scores, -1)
    context = weight @ enc_out                 # (B, P, D)

B=256, S=512, P=128, D=512 (fp32 I/O). Data-parallel over 8 NeuronCores
(32 batches per core). All matmuls fp16 on the PE (1 cycle/row vs 4 for
fp32; PSUM accumulates fp32); inputs cast to fp16 on host, output stored
fp16 and upcast on host.

Mask sparsity: masked positions get softmax weight exactly 0, so the
host gathers only the unmasked enc rows per batch (zero-padding to the
slot width w, a multiple of 8). Zero rows contribute exp(0-max) ~ e^-60
to the softmax denominator (invisible in fp32) and exactly 0 to the
context. Batches are sorted by unmasked count and dealt round-robin
across the 8 cores so one SPMD program serves all cores; output is
scattered back on host.

v3 structure:
  - Per-slot enc data packed into ONE dram tensor, per-partition layout
    [4*w enc^T (d-major)][(kt-1)*512 enc s-major full chunks][512
    remainder chunk], loaded as two contiguous DMAs (full-partition main
    rect on the sync queue, remainder rows on the scalar queue). No
    zero-padding traffic beyond pad-to-8.
  - Tails (weight transpose + context matmul + store) lag TWO slots
    behind the scores matmul, so the PE never waits on the
    mm2 -> reduce_max -> exp cross-engine chain. PE order per iter is
    transpose(b-2), mm2(b), mm3(b-2): the weight^T PSUM->SBUF copy
    (DVE) hides under mm2.
  - mm1 is spread one em-chain per iteration: group g+1's query^T is
    computed during group g's four iterations, one 4-matmul chain +
    one PSUM->SBUF cast each (casts alternate DVE/ACT), so no
    iteration carries a 2.7us cast burst.
  - PE warm-up: ~18 dummy matmuls on a memset tile raise the PE from
    its cold p-state (0.65/1.2 GHz) to 2.4 GHz while the first input
    DMAs are still in flight.
  - dect packed per-group contiguous ([128, 4*512] lines of 4KB),
    prefetched 6 iterations ahead; enc prefetched 3 slots ahead.

Per-core layout (K = PE contraction dim = partition dim):
  mm1  query^T (e,p): lhsT = W^T tiles (d,e) [stationary, shared],
       rhs = dec^T packed 4 slots (d, 4*128) -> N=512 moving.
  mm2  scores (p,s'): lhsT = query^T tiles, rhs = gathered enc^T tiles.
  softmax: DVE reduce_max (negate) -> ACT exp(bias=-max, accum_out=sum)
       -> DVE reciprocal; 1/sum applied by ACT during the context
       PSUM->SBUF copy (activation Copy, scale per partition).
  mm3  context (p,d): lhsT = weight^T (PE transposes), rhs = enc rows;
       last k-tile runs with K = remainder rows (never reads the
       unwritten pad partitions).
"""

import sys
import types

import numpy as np

B, SRC, PRED, D = 256, 512, 128, 512
N_CORES = 8
NB = B // N_CORES  # batches per core
TRIM_TAIL = True

MIN_W = 32
PREFETCH = 3  # enc slots in flight ahead of use
TAIL_LAG = 2
N_WARMUP = 18  # dummy matmuls to raise the PE p-state before real work


# ---------------------------------------------------------------------------
# environment shims (walrus 1-wait/instruction limit; missing axon hooks)
# ---------------------------------------------------------------------------
def _install_fixes():
    import concourse.tile as tile
    from concourse.tile import ScopedClock
    from concourse import mybir, bass_utils

    if not getattr(tile.TileContext, "_drain_split_installed", False):

        def _drain_and_barrier(self, tick_clock, wait_clock):
            nc = self.nc
            drain_inst = nc.sync.drain()
            wait_clock.add_sem_waits(
                drain_inst.ins, ScopedClock({None: tick_clock.global_clock})
            )
            waits = list(drain_inst.ins.sync_info.on_wait)
            if len(waits) > 1:
                drain_inst.ins.sync_info.on_wait = waits[:1]
                for w in waits[1:]:
                    extra = nc.sync.drain()
                    extra.ins.sync_info = mybir.SyncInfo(on_wait=[w], on_update=[])
            assert self.sems is not None
            popped = nc._tile_sem_poison_stack.pop()
            assert popped is self._sem_poison
            if not TRIM_TAIL:
                nc.all_engine_barrier()
                nc.clear_and_free_semaphores(list(self.sems.allocated().values()))
                nc.all_engine_barrier()
            # TRIM_TAIL: single execution per NEFF — skip the sem-clear
            # butterfly and barriers entirely (handles leak, harmless).

        tile.TileContext._drain_and_barrier = _drain_and_barrier
        tile.TileContext._drain_split_installed = True

    try:
        import antenv.axon_hooks  # noqa: F401
    except ImportError:
        try:
            if "/root/.axon_site" not in sys.path:
                sys.path.insert(0, "/root/.axon_site")
            from trn_agent_boot.trn_boot import _ntff_profile_via_ctypes

            hook = _ntff_profile_via_ctypes("/opt/axon/libaxon_pjrt.so")
            mod = types.ModuleType("antenv.axon_hooks")
            mod._hook = hook
            mod.get_axon_ntff_profile_hook = lambda: mod._hook
            mod.set_axon_ntff_profile_hook = lambda h: setattr(mod, "_hook", h)
            sys.modules["antenv.axon_hooks"] = mod
            import antenv

            antenv.axon_hooks = mod
        except Exception:
            pass

    bass_utils.upload_artifacts = lambda tmpdir: tmpdir

    # walrus in this image accepts only ONE sync-wait per instruction; Tile
    # emits several. Split extras onto EventSemaphore wait-carriers placed
    # just before the instruction in the same engine stream (JSON-level
    # post-pass on the serialized BIR).
    import json as _json
    import concourse.bass as _bass

    if not getattr(_bass.Bass, "_waitsplit_installed", False):
        _orig_to_json = _bass.Bass.to_json_bytes

        def _split_waits(bir: bytes) -> bytes:
            m = _json.loads(bir)
            ctr = 0
            changed = False
            for f in m["functions"]:
                for bb in f["blocks"]:
                    out = []
                    for inst in bb["instructions"]:
                        si = inst.get("sync_info")
                        waits = si.get("on_wait", []) if si else []
                        if len(waits) > 1:
                            changed = True
                            for w in waits[:-1]:
                                ctr += 1
                                out.append(
                                    {
                                        "debug": inst.get("debug", 0),
                                        "engine": inst["engine"],
                                        "ins": [],
                                        "outs": [],
                                        "name": f"waitsplit_{ctr}",
                                        "opcode": "EventSemaphore",
                                        "sync_info": {
                                            "on_update": [],
                                            "on_wait": [w],
                                        },
                                    }
                                )
                            si["on_wait"] = [waits[-1]]
                        out.append(inst)
                    bb["instructions"] = out
            if not changed:
                return bir
            return _json.dumps(m).encode()

        def to_json_bytes(self, *a, **k):
            return _split_waits(_orig_to_json(self, *a, **k))

        _bass.Bass.to_json_bytes = to_json_bytes
        _bass.Bass._waitsplit_installed = True


# ---------------------------------------------------------------------------
# slot planning: sort batches by unmasked count, deal across cores
# ---------------------------------------------------------------------------
def plan_slots(attn_mask, n_cores=N_CORES):
    """Returns (assigned, widths): assigned[i, c] = source batch index for
    core c slot i; widths[i] = padded-to-8 max unmasked count in slot i."""
    attn_mask = np.asarray(attn_mask)
    n = (~attn_mask).sum(axis=1)
    order = np.argsort(-n, kind="stable")
    nb = order.size // n_cores
    assigned = order.reshape(nb, n_cores)
    widths = []
    for i in range(nb):
        w = int(n[assigned[i]].max())
        w = min(SRC, max(MIN_W, ((w + 7) // 8) * 8))
        widths.append(w)
    return assigned, widths


def slot_geom(widths):
    """Per-slot (kt, rem, L): k-tile count, rows in the last k-tile, and
    packed per-partition length. Layout: [4*w enc^T][(kt-1)*512 enc
    full chunks][512 remainder chunk (rem rows used)]."""
    kts = [(w + 127) // 128 for w in widths]
    rems = [w - 128 * (kt - 1) for w, kt in zip(widths, kts)]
    Ls = [4 * w + kt * 512 for w, kt in zip(widths, kts)]
    return kts, rems, Ls


# ---------------------------------------------------------------------------
# bass program (one NeuronCore, NB slots with per-slot widths)
# ---------------------------------------------------------------------------
def build_bass(widths, nb=NB):
    import concourse.bass as bass
    import concourse.tile as tile
    from concourse import mybir, masks
    from contextlib import ExitStack

    assert len(widths) == nb
    kts, rems, Ls = slot_geom(widths)
    ktmax = max(kts)
    Lmax = max(Ls)

    f32 = mybir.dt.float32
    f16 = mybir.dt.float16
    nc = bass.Bass()

    comb_d = nc.dram_tensor("comb", [nb, 128, Lmax], f16, kind="ExternalInput")
    dect_d = nc.dram_tensor("dect", [nb // 4, 128, 4 * 512], f16, kind="ExternalInput")
    wts_d = nc.dram_tensor("wts", [128, 4 * D], f16, kind="ExternalInput")
    out_d = nc.dram_tensor("out", [nb, PRED, D], f16, kind="ExternalOutput")

    with tile.TileContext(nc) as tc, ExitStack() as ctx:
        const = ctx.enter_context(tc.tile_pool(name="const", bufs=1))
        enc_p = ctx.enter_context(
            tc.tile_pool(name="enc", bufs=PREFETCH + TAIL_LAG + 3)
        )
        dect_p = ctx.enter_context(tc.tile_pool(name="dect", bufs=3))
        qt_p = ctx.enter_context(tc.tile_pool(name="qt", bufs=2))
        w_p = ctx.enter_context(tc.tile_pool(name="w", bufs=TAIL_LAG + 2))
        wt_p = ctx.enter_context(tc.tile_pool(name="wt", bufs=2))
        o_p = ctx.enter_context(tc.tile_pool(name="o", bufs=4))
        st_p = ctx.enter_context(tc.tile_pool(name="st", bufs=2 * (TAIL_LAG + 1)))
        ps_qt = ctx.enter_context(
            tc.tile_pool(name="ps_qt", bufs=2, space=bass.MemorySpace.PSUM)
        )
        ps_tr = ctx.enter_context(
            tc.tile_pool(name="ps_tr", bufs=2, space=bass.MemorySpace.PSUM)
        )
        ps_sc = ctx.enter_context(
            tc.tile_pool(name="ps_sc", bufs=2, space=bass.MemorySpace.PSUM)
        )
        ps_cx = ctx.enter_context(
            tc.tile_pool(name="ps_cx", bufs=2, space=bass.MemorySpace.PSUM)
        )

        ident = const.tile([128, 128], f16)
        wts_sb = const.tile([128, 4 * D], f16)

        def load_enc(b):
            """One contiguous DMA per slot on the sync queue. The s-major
            section is transferred with its zero pad rows (already zero in
            DRAM): ~7% extra enc bytes, but the issue path has no
            cross-engine dependency that could block the queue head, and
            mm3 runs full-K tiles."""
            enc_sb = enc_p.tile([128, Lmax], f16, tag="enc")
            nc.sync.dma_start(enc_sb[:, 0 : Ls[b]], comb_d[b, :, 0 : Ls[b]])
            return enc_sb

        def load_dect(g):
            dect_sb = dect_p.tile([128, 4 * 512], f16, tag="dect", name="dect")
            nc.scalar.dma_start(dect_sb[:], dect_d[g])
            return dect_sb

        qt_tiles = {}
        dect_tiles = {}

        def emit_em(g, em):
            """One 4-matmul chain of group g's query^T + one PSUM cast."""
            q_ps = ps_qt.tile([128, 512], f32)
            for dk in range(4):
                nc.tensor.matmul(
                    q_ps[:],
                    wts_sb[:, dk * 512 + em * 128 : dk * 512 + (em + 1) * 128],
                    dect_tiles[g][:, dk * 512 : (dk + 1) * 512],
                    start=(dk == 0),
                    stop=(dk == 3),
                )
            dst = qt_tiles[g][:, em * 512 : (em + 1) * 512]
            if em % 2 == 0:
                nc.vector.tensor_copy(dst, q_ps[:])
            else:
                nc.scalar.activation(
                    dst, q_ps[:], mybir.ActivationFunctionType.Copy
                )

        def emit_tr(b, enc_sb, w_sb, recip):
            """weight^T via PE transposes + DVE copy back to SBUF."""
            kt, rem = kts[b], rems[b]
            wt_ps = ps_tr.tile([128, ktmax * 128], f16, tag="tr")
            for sk in range(kt):
                nc.tensor.transpose(
                    wt_ps[:, sk * 128 : (sk + 1) * 128],
                    w_sb[:, sk * 128 : (sk + 1) * 128],
                    ident[:],
                )
            wt_sb = wt_p.tile([128, ktmax * 128], f16, tag="wt")
            nc.vector.tensor_copy(wt_sb[:, 0 : kt * 128], wt_ps[:, 0 : kt * 128])
            return b, enc_sb, wt_sb, recip

        def emit_mm3(b, enc_sb, wt_sb, recip, last):
            kt, rem, w = kts[b], rems[b], widths[b]
            enc_off = 4 * w  # s-major chunks start here
            # last slot: split into halves so the first half's scale+store
            # overlaps the second half's matmuls (tail chain)
            halves = ((0, 256), (256, 512)) if last else ((0, 512),)
            o_sb = o_p.tile([128, D], f16, tag="o")
            for lo, hi in halves:
                cx_ps = ps_cx.tile([128, hi - lo], f32, tag="cx")
                for sk in range(kt):
                    nc.tensor.matmul(
                        cx_ps[:],
                        wt_sb[:, sk * 128 : (sk + 1) * 128],
                        enc_sb[:, enc_off + sk * 512 + lo : enc_off + sk * 512 + hi],
                        start=(sk == 0),
                        stop=(sk == kt - 1),
                    )
                nc.scalar.activation(
                    o_sb[:, lo:hi],
                    cx_ps[:],
                    mybir.ActivationFunctionType.Copy,
                    scale=recip[:],
                )
                nc.scalar.dma_start(out_d[b, :, lo:hi], o_sb[:, lo:hi])

        # startup: warm the PE p-state on dummy matmuls (on the weight
        # tile, first DMA to land) while the other operands stream in
        nc.sync.dma_start(wts_sb[:], wts_d[:])
        dect_tiles[0] = load_dect(0)  # scalar queue
        dect_tiles[1] = load_dect(1)
        enc_tiles = {i: load_enc(i) for i in range(min(PREFETCH, nb))}
        masks.make_identity(nc, ident[:])
        for _ in range(N_WARMUP):
            dum_ps = ps_cx.tile([128, 512], f32, tag="cx")
            nc.tensor.matmul(
                dum_ps[:], wts_sb[:, 0:128], wts_sb[:, 0:512],
                start=True, stop=True,
            )
        # group 0's query^T fully in the prologue
        qt_tiles[0] = qt_p.tile([128, 4 * 512], f16, tag="qt", name="qt")
        for em in range(4):
            emit_em(0, em)
        pend = []  # softmax-done slots awaiting their tail

        for b in range(nb):
            g, j = divmod(b, 4)
            w, kt = widths[b], kts[b]

            if b + PREFETCH < nb:
                enc_tiles[b + PREFETCH] = load_enc(b + PREFETCH)
            if j == 2 and g + 2 < nb // 4:
                dect_tiles[g + 2] = load_dect(g + 2)

            # ---- mm1 share: em chain j of group g+1 ----------------------
            if g + 1 < nb // 4:
                if j == 0:
                    qt_tiles[g + 1] = qt_p.tile([128, 4 * 512], f16, tag="qt", name="qt")
                emit_em(g + 1, j)

            # ---- weight^T of batch b-TAIL_LAG ----------------------------
            t_tr = None
            if len(pend) == TAIL_LAG:
                t_tr = emit_tr(*pend.pop(0))

            # ---- mm2: scores (p, s') -------------------------------------
            enc_sb = enc_tiles[b]
            sc_ps = ps_sc.tile([128, w], f32, tag="sc")
            for ek in range(4):
                nc.tensor.matmul(
                    sc_ps[:],
                    qt_tiles[g][:, ek * 512 + j * 128 : ek * 512 + (j + 1) * 128],
                    enc_sb[:, ek * w : (ek + 1) * w],
                    start=(ek == 0),
                    stop=(ek == 3),
                )

            # ---- softmax -------------------------------------------------
            negmax = st_p.tile([128, 1], f32, tag="negmax")
            nc.vector.reduce_max(
                negmax[:], sc_ps[:], axis=mybir.AxisListType.X, negate=True
            )
            w_sb = w_p.tile([128, ktmax * 128], f16, tag="w")
            if w < kt * 128:
                # zero pad cols: their transposed rows pair with zero enc rows
                nc.gpsimd.memset(w_sb[:, w : kt * 128], 0.0)
            sumexp = st_p.tile([128, 1], f32, tag="sumexp")
            nc.scalar.activation(
                w_sb[:, 0:w],
                sc_ps[:],
                mybir.ActivationFunctionType.Exp,
                bias=negmax[:],
                accum_out=sumexp[:],
            )
            recip = st_p.tile([128, 1], f32, tag="recip")
            nc.vector.reciprocal(recip[:], sumexp[:])

            # ---- context + store of batch b-TAIL_LAG ---------------------
            if t_tr is not None:
                emit_mm3(*t_tr, last=False)
                del enc_tiles[t_tr[0]]

            pend.append((b, enc_sb, w_sb, recip))

        while pend:
            t_tr = emit_tr(*pend.pop(0))
            emit_mm3(*t_tr, last=(not pend))
            del enc_tiles[t_tr[0]]

    return nc


# ---------------------------------------------------------------------------
# host-side sharding / gather
# ---------------------------------------------------------------------------
def prepare_in_maps(enc_out, dec_out, attn_mask, W, assigned, widths,
                    n_cores=N_CORES):
    enc_out = np.asarray(enc_out, dtype=np.float32)
    dec_out = np.asarray(dec_out, dtype=np.float32)
    attn_mask = np.asarray(attn_mask)
    W = np.asarray(W, dtype=np.float32)

    nb = assigned.shape[0]
    kts, rems, Ls = slot_geom(widths)
    Lmax = max(Ls)

    wt = W.T  # (d, e)
    wts = np.ascontiguousarray(
        wt.reshape(4, 128, D).transpose(1, 0, 2).reshape(128, 4 * D)
    ).astype(np.float16)

    enc16 = enc_out.astype(np.float16)
    in_maps = []
    for c in range(n_cores):
        idx = assigned[:, c]  # source batches in slot order
        comb = np.zeros((nb, 128, Lmax), dtype=np.float16)
        for i, src in enumerate(idx):
            rows = np.flatnonzero(~attn_mask[src])
            g = enc16[src, rows]  # (w0, 512)
            w0 = rows.size
            kt, w = kts[i], widths[i]
            # enc^T section: [p, ek*w + s] = g.T[ek*128+p, s]
            t = np.zeros((D, w), dtype=np.float16)
            t[:, :w0] = g.T
            comb[i, :, : 4 * w] = (
                t.reshape(4, 128, w).transpose(1, 0, 2).reshape(128, 4 * w)
            )
            # s-major section: [p, 4*w + sk*512 + e] = g[sk*128+p, e]
            gp = np.zeros((kt * 128, D), dtype=np.float16)
            gp[:w0] = g
            comb[i, :, 4 * w : 4 * w + kt * 512] = (
                gp.reshape(kt, 128, D).transpose(1, 0, 2).reshape(128, kt * 512)
            )
        dec_c = dec_out[idx]  # (nb, P, D)
        dect = np.ascontiguousarray(
            dec_c.reshape(nb // 4, 4, PRED, D)  # (g, jslot, p, d)
            .transpose(0, 3, 1, 2)              # (g, d, jslot, p)
            .reshape(nb // 4, 4, 128, 4 * PRED)  # (g, dk, dp, n)
            .transpose(0, 2, 1, 3)              # (g, dp, dk, n)
            .reshape(nb // 4, 128, 4 * 512)
        ).astype(np.float16)
        in_maps.append({"comb": comb, "dect": dect, "wts": wts})
    return in_maps


def run_sharded(enc_out, dec_out, attn_mask, W, trace=False, trace_kwargs=None):
    """Returns (full_output, BassKernelResults)."""
    _install_fixes()
    from concourse import bass_utils

    attn_mask = np.asarray(attn_mask)
    assigned, widths = plan_slots(attn_mask)
    nc = build_bass(widths)
    in_maps = prepare_in_maps(enc_out, dec_out, attn_mask, W, assigned, widths)
    res = bass_utils.run_bass_kernel_spmd(
        nc,
        in_maps,
        list(range(N_CORES)),
        trace=trace,
        **(trace_kwargs or {}),
    )
    out = np.empty((B, PRED, D), dtype=np.float32)
    for c in range(N_CORES):
        out[assigned[:, c]] = res.results[c]["out"].astype(np.float32)
    return out, res


def kernel(enc_out, dec_out, attn_mask, W):
    out, _ = run_sharded(enc_out, dec_out, attn_mask, W, trace=False)
    return out.astype(np.float32)


if __name__ == "__main__":
    print("building bass program...")
    _install_fixes()
    nc = build_bass([264] * NB)
    print("ok")


# revision 15
# speedup vs baseline: 1.1764x; 1.1350x over previous
"""Trainium2 Bass kernel for DecoderAttention (Luong attention).

reference:
    query   = dec_out @ W.T                    # (B, P, D)
    scores  = query @ enc_out.T (per batch)    # (B, P, S)
    scores  = where(mask, -inf, scores)
    weight  = softmax(scores, -1)
    context = weight @ enc_out                 # (B, P, D)

B=256, S=512, P=128, D=512 (fp32 I/O). Data-parallel over 8 NeuronCores
(32 batches per core). All matmuls fp16 on the PE (1 cycle/row vs 4 for
fp32; PSUM accumulates fp32); inputs cast to fp16 on host, output stored
fp16 and upcast on host.

Mask sparsity: masked positions get softmax weight exactly 0, so the
host gathers only the unmasked enc rows per batch (zero-padding to the
slot width w, a multiple of 8). Zero rows contribute exp(0-max) ~ e^-60
to the softmax denominator (invisible in fp32) and exactly 0 to the
context. Batches are sorted by unmasked count and dealt round-robin
across the 8 cores so one SPMD program serves all cores; output is
scattered back on host.

v3 structure:
  - Per-slot enc data packed into ONE dram tensor, per-partition layout
    [4*w enc^T (d-major)][(kt-1)*512 enc s-major full chunks][512
    remainder chunk], loaded as two contiguous DMAs (full-partition main
    rect on the sync queue, remainder rows on the scalar queue). No
    zero-padding traffic beyond pad-to-8.
  - Tails (weight transpose + context matmul + store) lag TWO slots
    behind the scores matmul, so the PE never waits on the
    mm2 -> reduce_max -> exp cross-engine chain. PE order per iter is
    transpose(b-2), mm2(b), mm3(b-2): the weight^T PSUM->SBUF copy
    (DVE) hides under mm2.
  - mm1 is spread one em-chain per iteration: group g+1's query^T is
    computed during group g's four iterations, one 4-matmul chain +
    one PSUM->SBUF cast each (casts alternate DVE/ACT), so no
    iteration carries a 2.7us cast burst.
  - PE warm-up: ~18 dummy matmuls on a memset tile raise the PE from
    its cold p-state (0.65/1.2 GHz) to 2.4 GHz while the first input
    DMAs are still in flight.
  - dect packed per-group contiguous ([128, 4*512] lines of 4KB),
    prefetched 6 iterations ahead; enc prefetched 3 slots ahead.

Per-core layout (K = PE contraction dim = partition dim):
  mm1  query^T (e,p): lhsT = W^T tiles (d,e) [stationary, shared],
       rhs = dec^T packed 4 slots (d, 4*128) -> N=512 moving.
  mm2  scores (p,s'): lhsT = query^T tiles, rhs = gathered enc^T tiles.
  softmax: DVE reduce_max (negate) -> ACT exp(bias=-max, accum_out=sum)
       -> DVE reciprocal; 1/sum applied by ACT during the context
       PSUM->SBUF copy (activation Copy, scale per partition).
  mm3  context (p,d): lhsT = weight^T (PE transposes), rhs = enc rows;
       last k-tile runs with K = remainder rows (never reads the
       unwritten pad partitions).
"""

import sys
import types

import numpy as np

B, SRC, PRED, D = 256, 512, 128, 512
N_CORES = 8
NB = B // N_CORES  # batches per core
TRIM_TAIL = True

MIN_W = 32
PREFETCH = 3  # enc slots in flight ahead of use
TAIL_LAG = 2
N_WARMUP = 18  # dummy matmuls to raise the PE p-state before real work


# ---------------------------------------------------------------------------
# environment shims (walrus 1-wait/instruction limit; missing axon hooks)
# ---------------------------------------------------------------------------
def _install_fixes():
    import concourse.tile as tile
    from concourse.tile import ScopedClock
    from concourse import mybir, bass_utils

    if not getattr(tile.TileContext, "_drain_split_installed", False):

        def _drain_and_barrier(self, tick_clock, wait_clock):
            nc = self.nc
            drain_inst = nc.sync.drain()
            wait_clock.add_sem_waits(
                drain_inst.ins, ScopedClock({None: tick_clock.global_clock})
            )
            waits = list(drain_inst.ins.sync_info.on_wait)
            if len(waits) > 1:
                drain_inst.ins.sync_info.on_wait = waits[:1]
                for w in waits[1:]:
                    extra = nc.sync.drain()
                    extra.ins.sync_info = mybir.SyncInfo(on_wait=[w], on_update=[])
            assert self.sems is not None
            popped = nc._tile_sem_poison_stack.pop()
            assert popped is self._sem_poison
            if not TRIM_TAIL:
                nc.all_engine_barrier()
                nc.clear_and_free_semaphores(list(self.sems.allocated().values()))
                nc.all_engine_barrier()
            # TRIM_TAIL: single execution per NEFF — skip the sem-clear
            # butterfly and barriers entirely (handles leak, harmless).

        tile.TileContext._drain_and_barrier = _drain_and_barrier
        tile.TileContext._drain_split_installed = True

    try:
        import antenv.axon_hooks  # noqa: F401
    except ImportError:
        try:
            if "/root/.axon_site" not in sys.path:
                sys.path.insert(0, "/root/.axon_site")
            from trn_agent_boot.trn_boot import _ntff_profile_via_ctypes

            hook = _ntff_profile_via_ctypes("/opt/axon/libaxon_pjrt.so")
            mod = types.ModuleType("antenv.axon_hooks")
            mod._hook = hook
            mod.get_axon_ntff_profile_hook = lambda: mod._hook
            mod.set_axon_ntff_profile_hook = lambda h: setattr(mod, "_hook", h)
            sys.modules["antenv.axon_hooks"] = mod
            import antenv

            antenv.axon_hooks = mod
        except Exception:
            pass

    bass_utils.upload_artifacts = lambda tmpdir: tmpdir

    # walrus in this image accepts only ONE sync-wait per instruction; Tile
    # emits several. Split extras onto EventSemaphore wait-carriers placed
    # just before the instruction in the same engine stream (JSON-level
    # post-pass on the serialized BIR).
    import json as _json
    import concourse.bass as _bass

    if not getattr(_bass.Bass, "_waitsplit_installed", False):
        _orig_to_json = _bass.Bass.to_json_bytes

        def _split_waits(bir: bytes) -> bytes:
            m = _json.loads(bir)
            ctr = 0
            changed = False
            for f in m["functions"]:
                for bb in f["blocks"]:
                    out = []
                    for inst in bb["instructions"]:
                        si = inst.get("sync_info")
                        waits = si.get("on_wait", []) if si else []
                        if len(waits) > 1:
                            changed = True
                            for w in waits[:-1]:
                                ctr += 1
                                out.append(
                                    {
                                        "debug": inst.get("debug", 0),
                                        "engine": inst["engine"],
                                        "ins": [],
                                        "outs": [],
                                        "name": f"waitsplit_{ctr}",
                                        "opcode": "EventSemaphore",
                                        "sync_info": {
                                            "on_update": [],
                                            "on_wait": [w],
                                        },
                                    }
                                )
                            si["on_wait"] = [waits[-1]]
                        out.append(inst)
                    bb["instructions"] = out
            if not changed:
                return bir
            return _json.dumps(m).encode()

        def to_json_bytes(self, *a, **k):
            return _split_waits(_orig_to_json(self, *a, **k))

        _bass.Bass.to_json_bytes = to_json_bytes
        _bass.Bass._waitsplit_installed = True


# ---------------------------------------------------------------------------
# slot planning: sort batches by unmasked count, deal across cores
# ---------------------------------------------------------------------------
def plan_slots(attn_mask, n_cores=N_CORES):
    """Returns (assigned, widths): assigned[i, c] = source batch index for
    core c slot i; widths[i] = padded-to-8 max unmasked count in slot i."""
    attn_mask = np.asarray(attn_mask)
    n = (~attn_mask).sum(axis=1)
    order = np.argsort(-n, kind="stable")
    nb = order.size // n_cores
    assigned = order.reshape(nb, n_cores)
    widths = []
    for i in range(nb):
        w = int(n[assigned[i]].max())
        w = min(SRC, max(MIN_W, ((w + 7) // 8) * 8))
        widths.append(w)
    return assigned, widths


def slot_geom(widths):
    """Per-slot (kt, rem, L): k-tile count, rows in the last k-tile, and
    packed per-partition length. Layout: [4*w enc^T][(kt-1)*512 enc
    full chunks][512 remainder chunk (rem rows used)]."""
    kts = [(w + 127) // 128 for w in widths]
    rems = [w - 128 * (kt - 1) for w, kt in zip(widths, kts)]
    Ls = [4 * w + kt * 512 for w, kt in zip(widths, kts)]
    return kts, rems, Ls


# ---------------------------------------------------------------------------
# bass program (one NeuronCore, NB slots with per-slot widths)
# ---------------------------------------------------------------------------
def build_bass(widths, nb=NB):
    import concourse.bass as bass
    import concourse.tile as tile
    from concourse import mybir, masks
    from contextlib import ExitStack

    assert len(widths) == nb
    kts, rems, Ls = slot_geom(widths)
    ktmax = max(kts)
    Lmax = max(Ls)

    f32 = mybir.dt.float32
    f16 = mybir.dt.float16
    nc = bass.Bass()

    comb_d = nc.dram_tensor("comb", [nb, 128, Lmax], f16, kind="ExternalInput")
    dect_d = nc.dram_tensor("dect", [nb // 4, 128, 4 * 512], f16, kind="ExternalInput")
    wts_d = nc.dram_tensor("wts", [128, 4 * D], f16, kind="ExternalInput")
    out_d = nc.dram_tensor("out", [nb, PRED, D], f16, kind="ExternalOutput")

    with tile.TileContext(nc) as tc, ExitStack() as ctx:
        const = ctx.enter_context(tc.tile_pool(name="const", bufs=1))
        enc_p = ctx.enter_context(
            tc.tile_pool(name="enc", bufs=PREFETCH + TAIL_LAG + 3)
        )
        dect_p = ctx.enter_context(tc.tile_pool(name="dect", bufs=4))
        qt_p = ctx.enter_context(tc.tile_pool(name="qt", bufs=2))
        w_p = ctx.enter_context(tc.tile_pool(name="w", bufs=TAIL_LAG + 2))
        wt_p = ctx.enter_context(tc.tile_pool(name="wt", bufs=2))
        o_p = ctx.enter_context(tc.tile_pool(name="o", bufs=4))
        st_p = ctx.enter_context(tc.tile_pool(name="st", bufs=2 * (TAIL_LAG + 1)))
        ps_qt = ctx.enter_context(
            tc.tile_pool(name="ps_qt", bufs=2, space=bass.MemorySpace.PSUM)
        )
        ps_tr = ctx.enter_context(
            tc.tile_pool(name="ps_tr", bufs=2, space=bass.MemorySpace.PSUM)
        )
        ps_sc = ctx.enter_context(
            tc.tile_pool(name="ps_sc", bufs=2, space=bass.MemorySpace.PSUM)
        )
        ps_cx = ctx.enter_context(
            tc.tile_pool(name="ps_cx", bufs=2, space=bass.MemorySpace.PSUM)
        )

        ident = const.tile([128, 128], f16)
        wts_sb = const.tile([128, 4 * D], f16)

        def load_enc(b):
            """One contiguous DMA per slot on the sync queue. The s-major
            section is transferred with its zero pad rows (already zero in
            DRAM): ~7% extra enc bytes, but the issue path has no
            cross-engine dependency that could block the queue head, and
            mm3 runs full-K tiles."""
            enc_sb = enc_p.tile([128, Lmax], f16, tag="enc")
            nc.sync.dma_start(enc_sb[:, 0 : Ls[b]], comb_d[b, :, 0 : Ls[b]])
            return enc_sb

        def load_dect(g):
            # sync queue: rides between enc mains in the same ring, so it
            # cannot starve behind them (the scalar ring can)
            dect_sb = dect_p.tile([128, 4 * 512], f16, tag="dect", name="dect")
            nc.sync.dma_start(dect_sb[:], dect_d[g])
            return dect_sb

        qt_tiles = {}
        dect_tiles = {}

        def emit_em(g, em):
            """One 4-matmul chain of group g's query^T + one PSUM cast."""
            q_ps = ps_qt.tile([128, 512], f32)
            for dk in range(4):
                nc.tensor.matmul(
                    q_ps[:],
                    wts_sb[:, dk * 512 + em * 128 : dk * 512 + (em + 1) * 128],
                    dect_tiles[g][:, dk * 512 : (dk + 1) * 512],
                    start=(dk == 0),
                    stop=(dk == 3),
                )
            dst = qt_tiles[g][:, em * 512 : (em + 1) * 512]
            if em % 2 == 0:
                nc.vector.tensor_copy(dst, q_ps[:])
            else:
                nc.scalar.activation(
                    dst, q_ps[:], mybir.ActivationFunctionType.Copy
                )

        def emit_tr(b, enc_sb, w_sb, recip):
            """weight^T via PE transposes + DVE copy back to SBUF."""
            kt, rem = kts[b], rems[b]
            wt_ps = ps_tr.tile([128, ktmax * 128], f16, tag="tr")
            for sk in range(kt):
                nc.tensor.transpose(
                    wt_ps[:, sk * 128 : (sk + 1) * 128],
                    w_sb[:, sk * 128 : (sk + 1) * 128],
                    ident[:],
                )
            wt_sb = wt_p.tile([128, ktmax * 128], f16, tag="wt")
            nc.vector.tensor_copy(wt_sb[:, 0 : kt * 128], wt_ps[:, 0 : kt * 128])
            return b, enc_sb, wt_sb, recip

        def emit_mm3(b, enc_sb, wt_sb, recip, last):
            kt, rem, w = kts[b], rems[b], widths[b]
            enc_off = 4 * w  # s-major chunks start here
            # last slot: split into halves so the first half's scale+store
            # overlaps the second half's matmuls (tail chain)
            halves = ((0, 256), (256, 512)) if last else ((0, 512),)
            o_sb = o_p.tile([128, D], f16, tag="o")
            for lo, hi in halves:
                cx_ps = ps_cx.tile([128, hi - lo], f32, tag="cx")
                for sk in range(kt):
                    nc.tensor.matmul(
                        cx_ps[:],
                        wt_sb[:, sk * 128 : (sk + 1) * 128],
                        enc_sb[:, enc_off + sk * 512 + lo : enc_off + sk * 512 + hi],
                        start=(sk == 0),
                        stop=(sk == kt - 1),
                    )
                nc.scalar.activation(
                    o_sb[:, lo:hi],
                    cx_ps[:],
                    mybir.ActivationFunctionType.Copy,
                    scale=recip[:],
                )
                nc.scalar.dma_start(out_d[b, :, lo:hi], o_sb[:, lo:hi])

        # startup: warm the PE p-state on dummy matmuls (on the weight
        # tile, first DMA to land) while the other operands stream in
        nc.sync.dma_start(wts_sb[:], wts_d[:])
        dect_tiles[0] = load_dect(0)  # scalar queue
        dect_tiles[1] = load_dect(1)
        enc_tiles = {i: load_enc(i) for i in range(min(PREFETCH, nb))}
        masks.make_identity(nc, ident[:])
        for _ in range(N_WARMUP):
            dum_ps = ps_cx.tile([128, 512], f32, tag="cx")
            nc.tensor.matmul(
                dum_ps[:], wts_sb[:, 0:128], wts_sb[:, 0:512],
                start=True, stop=True,
            )
        # group 0's query^T fully in the prologue
        qt_tiles[0] = qt_p.tile([128, 4 * 512], f16, tag="qt", name="qt")
        for em in range(4):
            emit_em(0, em)
        pend = []  # softmax-done slots awaiting their tail

        for b in range(nb):
            g, j = divmod(b, 4)
            w, kt = widths[b], kts[b]

            if b + PREFETCH < nb:
                enc_tiles[b + PREFETCH] = load_enc(b + PREFETCH)
            if j == 0 and g + 2 < nb // 4:
                dect_tiles[g + 2] = load_dect(g + 2)

            # ---- mm1 share: em chain j of group g+1 ----------------------
            if g + 1 < nb // 4:
                if j == 0:
                    qt_tiles[g + 1] = qt_p.tile([128, 4 * 512], f16, tag="qt", name="qt")
                emit_em(g + 1, j)

            # ---- weight^T of batch b-TAIL_LAG ----------------------------
            t_tr = None
            if len(pend) == TAIL_LAG:
                t_tr = emit_tr(*pend.pop(0))

            # ---- mm2: scores (p, s') -------------------------------------
            enc_sb = enc_tiles[b]
            sc_ps = ps_sc.tile([128, w], f32, tag="sc")
            for ek in range(4):
                nc.tensor.matmul(
                    sc_ps[:],
                    qt_tiles[g][:, ek * 512 + j * 128 : ek * 512 + (j + 1) * 128],
                    enc_sb[:, ek * w : (ek + 1) * w],
                    start=(ek == 0),
                    stop=(ek == 3),
                )

            # ---- softmax -------------------------------------------------
            negmax = st_p.tile([128, 1], f32, tag="negmax")
            nc.vector.reduce_max(
                negmax[:], sc_ps[:], axis=mybir.AxisListType.X, negate=True
            )
            w_sb = w_p.tile([128, ktmax * 128], f16, tag="w")
            if w < kt * 128:
                # zero pad cols: their transposed rows pair with zero enc rows
                nc.gpsimd.memset(w_sb[:, w : kt * 128], 0.0)
            sumexp = st_p.tile([128, 1], f32, tag="sumexp")
            nc.scalar.activation(
                w_sb[:, 0:w],
                sc_ps[:],
                mybir.ActivationFunctionType.Exp,
                bias=negmax[:],
                accum_out=sumexp[:],
            )
            recip = st_p.tile([128, 1], f32, tag="recip")
            nc.vector.reciprocal(recip[:], sumexp[:])

            # ---- context + store of batch b-TAIL_LAG ---------------------
            if t_tr is not None:
                emit_mm3(*t_tr, last=False)
                del enc_tiles[t_tr[0]]

            pend.append((b, enc_sb, w_sb, recip))

        while pend:
            t_tr = emit_tr(*pend.pop(0))
            emit_mm3(*t_tr, last=(not pend))
            del enc_tiles[t_tr[0]]

    return nc


# ---------------------------------------------------------------------------
# host-side sharding / gather
# ---------------------------------------------------------------------------
def prepare_in_maps(enc_out, dec_out, attn_mask, W, assigned, widths,
                    n_cores=N_CORES):
    enc_out = np.asarray(enc_out, dtype=np.float32)
    dec_out = np.asarray(dec_out, dtype=np.float32)
    attn_mask = np.asarray(attn_mask)
    W = np.asarray(W, dtype=np.float32)

    nb = assigned.shape[0]
    kts, rems, Ls = slot_geom(widths)
    Lmax = max(Ls)

    wt = W.T  # (d, e)
    wts = np.ascontiguousarray(
        wt.reshape(4, 128, D).transpose(1, 0, 2).reshape(128, 4 * D)
    ).astype(np.float16)

    enc16 = enc_out.astype(np.float16)
    in_maps = []
    for c in range(n_cores):
        idx = assigned[:, c]  # source batches in slot order
        comb = np.zeros((nb, 128, Lmax), dtype=np.float16)
        for i, src in enumerate(idx):
            rows = np.flatnonzero(~attn_mask[src])
            g = enc16[src, rows]  # (w0, 512)
            w0 = rows.size
            kt, w = kts[i], widths[i]
            # enc^T section: [p, ek*w + s] = g.T[ek*128+p, s]
            t = np.zeros((D, w), dtype=np.float16)
            t[:, :w0] = g.T
            comb[i, :, : 4 * w] = (
                t.reshape(4, 128, w).transpose(1, 0, 2).reshape(128, 4 * w)
            )
            # s-major section: [p, 4*w + sk*512 + e] = g[sk*128+p, e]
            gp = np.zeros((kt * 128, D), dtype=np.float16)
            gp[:w0] = g
            comb[i, :, 4 * w : 4 * w + kt * 512] = (
                gp.reshape(kt, 128, D).transpose(1, 0, 2).reshape(128, kt * 512)
            )
        dec_c = dec_out[idx]  # (nb, P, D)
        dect = np.ascontiguousarray(
            dec_c.reshape(nb // 4, 4, PRED, D)  # (g, jslot, p, d)
            .transpose(0, 3, 1, 2)              # (g, d, jslot, p)
            .reshape(nb // 4, 4, 128, 4 * PRED)  # (g, dk, dp, n)
            .transpose(0, 2, 1, 3)              # (g, dp, dk, n)
            .reshape(nb // 4, 128, 4 * 512)
        ).astype(np.float16)
        in_maps.append({"comb": comb, "dect": dect, "wts": wts})
    return in_maps


def run_sharded(enc_out, dec_out, attn_mask, W, trace=False, trace_kwargs=None):
    """Returns (full_output, BassKernelResults)."""
    _install_fixes()
    from concourse import bass_utils

    attn_mask = np.asarray(attn_mask)
    assigned, widths = plan_slots(attn_mask)
    nc = build_bass(widths)
    in_maps = prepare_in_maps(enc_out, dec_out, attn_mask, W, assigned, widths)
    res = bass_utils.run_bass_kernel_spmd(
        nc,
        in_maps,
        list(range(N_CORES)),
        trace=trace,
        **(trace_kwargs or {}),
    )
    out = np.empty((B, PRED, D), dtype=np.float32)
    for c in range(N_CORES):
        out[assigned[:, c]] = res.results[c]["out"].astype(np.float32)
    return out, res


def kernel(enc_out, dec_out, attn_mask, W):
    out, _ = run_sharded(enc_out, dec_out, attn_mask, W, trace=False)
    return out.astype(np.float32)


if __name__ == "__main__":
    print("building bass program...")
    _install_fixes()
    nc = build_bass([264] * NB)
    print("ok")
